# revision 1
# baseline (speedup 1.0000x reference)
"""DisenConv (disentangled GNN message passing) Trainium2 kernel.

Strategy (8 NeuronCores, no collectives):
  - Nodes split into 8 contiguous ranges; core c owns range c. Edges are
    assigned to the core owning their TARGET node and sorted by target, so
    both the u[trg] gather and the segment-sum scatter are core-local.
  - Per core, targets are grouped into 128-node blocks; each block's edges are
    padded into chunks of 128 edges. Gather and scatter both become TensorE
    matmuls against per-chunk one-hot matrices (fp8, built on-device from
    run-length metadata).
  - The host shards x[src] per edge (input sharding only); the device
    computes z = rownorm(x_src) directly (rownorm(x)[src] == rownorm(x[src]))
    and streams z (fp16) from DRAM each iteration.
  - Per-iteration work: gather-mms -> logits (DVE mult + grouped reduce) ->
    softmax (ACT exp + DVE) -> msg (DVE) -> scatter-mms -> per-4-block-group
    add-xk + L2 normalize, updating a resident fp16 u in SBUF.

kernel(**inputs) takes the FULL inputs and returns the FULL output.
"""
import math
from dataclasses import dataclass, field

import numpy as np

P = 128          # partitions / block size / chunk size
K = 8            # latent factors
NITER = 6
EPS2 = 1e-24     # sqrt(ss + EPS2) ~= max(sqrt(ss), 1e-12)


# ----------------------------------------------------------------------------
# host-side layout
# ----------------------------------------------------------------------------

@dataclass
class CoreLayout:
    src_idx: np.ndarray    # [P, C_tot] int32  source node of edge (chunk, lane); pad -> 0
    trgrel: np.ndarray     # [P, C_tot] float32 (bf16-exact) target-in-block; pad -> -1
    starts: np.ndarray     # [P, C_tot] float32 run starts per (t, chunk)
    ends: np.ndarray       # [P, C_tot] float32 run ends per (t, chunk)
    slot_idx: np.ndarray   # [P, NB] int32  node id of (slot block, lane); pad -> n_nodes
    out_rows: np.ndarray   # [n_loc] int32  u_out row of local node i


@dataclass
class Layout:
    n_nodes: int
    n_cores: int
    n_loc: int
    NB: int                # block slots per core
    caps: list             # chunks per slot (uniform across cores)
    cum: np.ndarray
    C_tot: int
    G: int                 # chunks per z tile
    NT: int                # z tiles
    cores: list = field(default_factory=list)


def build_layout(edge_index: np.ndarray, n_nodes: int, n_cores: int, G: int = 32) -> Layout:
    src = edge_index[0].astype(np.int64)
    trg = edge_index[1].astype(np.int64)
    assert n_nodes % n_cores == 0
    n_loc = n_nodes // n_cores
    NB = math.ceil(n_loc / P)

    per_core = []
    counts_all = np.zeros((n_cores, NB), dtype=np.int64)
    for c in range(n_cores):
        lo, hi = c * n_loc, (c + 1) * n_loc
        m = (trg >= lo) & (trg < hi)
        es, et = src[m], trg[m] - lo
        order = np.argsort(et, kind="stable")
        es, et = es[order], et[order]
        counts_all[c] = np.bincount(et // P, minlength=NB)
        per_core.append((es, et))

    orders = [np.argsort(-counts_all[c], kind="stable") for c in range(n_cores)]
    sorted_counts = np.stack([counts_all[c][orders[c]] for c in range(n_cores)])
    caps = np.maximum(1, np.ceil(sorted_counts.max(axis=0) / P).astype(np.int64))
    C_tot = int(caps.sum())
    pad = (-C_tot) % G
    caps[-1] += pad
    C_tot += pad
    cum = np.concatenate([[0], np.cumsum(caps)])[:-1]

    lay = Layout(n_nodes=n_nodes, n_cores=n_cores, n_loc=n_loc, NB=NB,
                 caps=[int(x) for x in caps], cum=cum, C_tot=C_tot, G=G, NT=C_tot // G)

    for c in range(n_cores):
        es, et = per_core[c]
        order_c = orders[c]
        slot_of_block = np.empty(NB, dtype=np.int64)
        slot_of_block[order_c] = np.arange(NB)

        src_idx = np.zeros((P, C_tot), dtype=np.int32)
        trgrel = np.full((P, C_tot), -1.0, dtype=np.float32)
        starts = np.zeros((P, C_tot), dtype=np.float32)
        ends = np.zeros((P, C_tot), dtype=np.float32)

        bstart = np.concatenate([[0], np.cumsum(counts_all[c])])
        for s in range(NB):
            b = order_c[s]
            e0, e1 = bstart[b], bstart[b + 1]
            n_e = e1 - e0
            if n_e == 0:
                continue
            tr = (et[e0:e1] - b * P).astype(np.int64)
            c0 = lay.cum[s]
            j = np.arange(n_e)
            src_idx[j % P, c0 + j // P] = es[e0:e1]
            trgrel[j % P, c0 + j // P] = tr
            for q in range(int(math.ceil(n_e / P))):
                t_chunk = tr[q * P:(q + 1) * P]
                cnts = np.bincount(t_chunk, minlength=P)
                e_run = np.cumsum(cnts)
                starts[:, c0 + q] = e_run - cnts
                ends[:, c0 + q] = e_run

        slot_idx = np.full((P, NB), n_nodes, dtype=np.int32)
        for s in range(NB):
            b = order_c[s]
            base = c * n_loc + b * P
            n_in = min(P, n_loc - b * P)
            slot_idx[:n_in, s] = base + np.arange(n_in)

        i = np.arange(n_loc)
        out_rows = (slot_of_block[i // P] * P + (i % P)).astype(np.int32)

        lay.cores.append(CoreLayout(src_idx=src_idx, trgrel=trgrel, starts=starts,
                                    ends=ends, slot_idx=slot_idx, out_rows=out_rows))
    return lay


# ----------------------------------------------------------------------------
# numpy model of exactly what the device computes (for validation)
# ----------------------------------------------------------------------------

def numpy_model(x: np.ndarray, edge_index: np.ndarray, n_cores: int, niter: int = NITER,
                fp16: bool = True) -> np.ndarray:
    f16 = np.float16 if fp16 else np.float32
    n, d = x.shape
    dd = d // K
    lay = build_layout(edge_index, n, n_cores)

    def rownorm(v):
        ss = (v.reshape(-1, K, dd) ** 2).sum(axis=2)
        rs = 1.0 / np.sqrt(ss + EPS2)
        return (v.reshape(-1, K, dd) * rs[:, :, None]).reshape(-1, d)

    xb = np.vstack([x, np.ones((1, d), dtype=np.float32)])
    out = np.zeros((n, d), dtype=np.float32)
    for c in range(n_cores):
        cl = lay.cores[c]
        z = rownorm(x[cl.src_idx.T.reshape(-1)]).astype(f16)
        xkl = rownorm(xb[cl.slot_idx.T.reshape(-1)]).astype(np.float32)
        u = xkl.astype(f16)
        trgrel = cl.trgrel.T.reshape(-1).astype(np.int64)
        valid = trgrel >= 0
        ch_slot = np.zeros(lay.C_tot, dtype=np.int64)
        for s in range(lay.NB):
            ch_slot[lay.cum[s]:lay.cum[s] + lay.caps[s]] = s
        glob_t = ch_slot.repeat(P) * P + np.where(valid, trgrel, 0)
        uo = None
        for it in range(niter):
            ut = u[glob_t] * valid[:, None].astype(f16)
            w = (z * ut).astype(f16)
            logits = w.reshape(-1, K, dd).astype(np.float32).sum(axis=2)
            expt = np.exp(logits).astype(f16)
            s_ = expt.astype(np.float32).reshape(-1, K).sum(axis=1)
            rqv = (1.0 / s_).astype(np.float16)
            expn = (expt.astype(np.float32) * rqv[:, None].astype(np.float32)).astype(f16)
            msg = (z.astype(np.float32) * expn.astype(np.float32).repeat(dd, axis=1)).astype(f16)
            agg = np.zeros((lay.NB * P, d), dtype=np.float32)
            np.add.at(agg, glob_t[valid], msg[valid].astype(np.float32))
            t0 = agg + xkl
            ssn = (t0.reshape(-1, K, dd) ** 2).sum(axis=2)
            rsn = 1.0 / np.sqrt(ssn + EPS2)
            un = t0.reshape(-1, K, dd) * rsn[:, :, None]
            u = un.reshape(-1, d).astype(f16)
            uo = un.reshape(-1, d).astype(np.float32)
        lo = c * lay.n_loc
        out[lo:lo + lay.n_loc] = uo[cl.out_rows]
    return out


# ----------------------------------------------------------------------------
# bass kernel emitter
# ----------------------------------------------------------------------------

def _swap_last2(ap):
    """Swap the last two free dims of an AP (strided view, no data movement)."""
    import concourse.bass as bass
    lst = [list(dd) for dd in ap.ap]
    lst[-1], lst[-2] = lst[-2], lst[-1]
    return bass.AP(ap.tensor, ap.offset, lst)


def _mid_bcast(ap, pos, count):
    """Insert a step-0 dim of size `count` at position `pos` of the AP."""
    import concourse.bass as bass
    lst = [list(dd) for dd in ap.ap]
    lst.insert(pos, [0, count])
    return bass.AP(ap.tensor, ap.offset, lst)


def build_nc(lay: Layout, niter: int = NITER, d: int = 128):
    import contextlib
    import concourse.bass as bass
    import concourse.mybir as mybir
    import concourse.tile as tile
    import bass_rust as _bass_rust

    f32, f16, bf16, fp8 = (mybir.dt.float32, mybir.dt.float16,
                           mybir.dt.bfloat16, mybir.dt.float8e4)
    i16 = mybir.dt.int16
    Alu = mybir.AluOpType
    Act = mybir.ActivationFunctionType
    X = mybir.AxisListType.X
    dd = d // K

    G, C_tot, NT, NB = lay.G, lay.C_tot, lay.NT, lay.NB
    NBG = math.ceil(NB / 4)

    ch_slot = np.zeros(C_tot, dtype=np.int64)
    for s in range(NB):
        ch_slot[lay.cum[s]:lay.cum[s] + lay.caps[s]] = s
    slot_first = {int(lay.cum[s]): s for s in range(NB)}
    slot_last = {int(lay.cum[s] + lay.caps[s] - 1): s for s in range(NB)}

    nc = bass.Bass()
    x_src = nc.dram_tensor("x_src", [NT, P, G * d], f16, kind="ExternalInput")
    x_local = nc.dram_tensor("x_local", [NBG, P, 4 * d], f32, kind="ExternalInput")
    trgrel = nc.dram_tensor("trgrel", [P, C_tot], bf16, kind="ExternalInput")
    starts = nc.dram_tensor("starts", [P, C_tot], bf16, kind="ExternalInput")
    ends = nc.dram_tensor("ends", [P, C_tot], bf16, kind="ExternalInput")
    u_out = nc.dram_tensor("u_out", [NBG * 4 * P, d], f32, kind="ExternalOutput")

    z = nc.dram_tensor("z", [NT, P, G * d], f16, kind="Internal")
    xkl = nc.dram_tensor("xkl", [NBG, P, 4 * d], f32, kind="Internal")
    et_dram = nc.dram_tensor("et_dram", [NT, P, G * P], f16, kind="Internal")
    te_dram = nc.dram_tensor("te_dram", [NT, P, G * P], f16, kind="Internal")

    with tile.TileContext(nc) as tc:
        with contextlib.ExitStack() as ctx:
            cpool = ctx.enter_context(tc.tile_pool(name="const", bufs=1))
            wpool = ctx.enter_context(tc.tile_pool(name="work", bufs=2))
            ppool = ctx.enter_context(tc.tile_pool(name="psum", bufs=1, space="PSUM"))
            apool = ctx.enter_context(tc.tile_pool(name="aggp", bufs=3, space="PSUM"))

            meta_tr = cpool.tile([P, C_tot], bf16, tag="mtr")
            meta_st = cpool.tile([P, C_tot], bf16, tag="mst")
            meta_en = cpool.tile([P, C_tot], bf16, tag="men")
            iota_bf = cpool.tile([P, P], bf16, tag="iota")
            epst = cpool.tile([P, 1], f32, tag="epst")
            u_g = [cpool.tile([P, 4 * d], f16, tag=f"u{g}", name=f"u{g}")
                   for g in range(NBG)]

            nc.sync.dma_start(out=meta_tr[:], in_=trgrel[:])
            nc.sync.dma_start(out=meta_st[:], in_=starts[:])
            nc.sync.dma_start(out=meta_en[:], in_=ends[:])
            nc.vector.memset(epst[:], EPS2)
            iota_i16 = wpool.tile([P, P], i16, tag="ioti")
            nc.gpsimd.iota(iota_i16[:], pattern=[[1, P]], base=0, channel_multiplier=0)
            nc.vector.tensor_copy(out=iota_bf[:], in_=iota_i16[:])

            def rownorm_emit(src_ap, out_ap, nrows, tags, sq_bufs=2, sq_dt=f32, rdt=f32):
                """out_ap = per-factor L2 normalize of src_ap [P, nrows*d]."""
                sq = wpool.tile([P, nrows * d], sq_dt, tag=tags[0], name=f"sq_{tags[0]}", bufs=sq_bufs)
                nc.vector.tensor_tensor(out=sq[:, :nrows * d], in0=src_ap, in1=src_ap,
                                        op=Alu.mult)
                ssn = wpool.tile([P, nrows * K], f32, tag=tags[1], name=f"ss_{tags[1]}")
                nc.vector.tensor_reduce(
                    out=ssn[:, :nrows * K],
                    in_=_swap_last2(sq[:, :nrows * d].rearrange(
                        "p (r s k) -> p r s k", s=dd, k=K)),
                    axis=X, op=Alu.add)
                nrm = wpool.tile([P, nrows * K], f32, tag=tags[2], name=f"nr_{tags[2]}")
                nc.scalar.activation(nrm[:, :nrows * K], ssn[:, :nrows * K], Act.Sqrt,
                                     bias=epst[:])
                rsn = wpool.tile([P, nrows * K], rdt, tag=tags[3], name=f"rs_{tags[3]}")
                with nc.allow_low_precision(reason="unit-scale reciprocal"):
                    nc.vector.reciprocal(out=rsn[:, :nrows * K], in_=nrm[:, :nrows * K])
                nc.vector.tensor_tensor(
                    out=out_ap.rearrange("p (r s k) -> p r s k", s=dd, k=K),
                    in0=src_ap.rearrange("p (r s k) -> p r s k", s=dd, k=K),
                    in1=_mid_bcast(rsn[:, :nrows * K].rearrange(
                        "p (r k) -> p r k", k=K), 2, dd),
                    op=Alu.mult)

            # ---- phase 0: xk_local = rownorm(x_local) -> DRAM; u init
            xkl_writes = {}
            for g in range(NBG):
                stg = wpool.tile([P, 4 * d], f32, tag="xklst")
                nc.sync.dma_start(out=stg[:], in_=x_local[g])
                xlt = wpool.tile([P, 4 * d], f32, tag="t0")
                rownorm_emit(stg[:], xlt[:], 4, ("xkb", "nss", "nnr", "nrs"))
                xkl_writes[g] = nc.sync.dma_start(out=xkl[g], in_=xlt[:])
                nc.vector.tensor_copy(out=u_g[g][:], in_=xlt[:])

            # ---- phase 1+2 interleaved: z = rownorm(x_src); one-hots
            z_writes = {}
            et_writes = {}
            te_writes = {}
            HS = G * d // 2
            for t in range(NT):
                zb = wpool.tile([P, G * d], f16, tag="zbuf", bufs=3)
                for h in range(2):
                    xs = wpool.tile([P, HS], f16, tag="w", name="xs", bufs=3)
                    nc.sync.dma_start(out=xs[:], in_=x_src[t][:, h * HS:(h + 1) * HS])
                    rownorm_emit(xs[:], zb[:, h * HS:(h + 1) * HS], G // 2,
                                 ("msg", "ssz", "nrz", "rsz"), sq_bufs=3,
                                 sq_dt=f16, rdt=f16)
                z_writes[t] = nc.sync.dma_start(out=z[t], in_=zb[:])
                # gather one-hot segment for this tile's chunks (resident fp8)
                s0, ns = t * G, G
                A = wpool.tile([P, G * P], f16, tag="utrg", name="ohA", bufs=3)
                B = wpool.tile([P, G * P], f16, tag="msg", name="ohB", bufs=3)
                iota_ce = _mid_bcast(iota_bf[:], 1, ns)
                nc.vector.tensor_tensor(out=A[:, :ns * P].rearrange("p (c e) -> p c e", e=P),
                                        in0=iota_ce,
                                        in1=meta_st[:, s0:s0 + ns].broadcast_to([P, ns, P]),
                                        op=Alu.is_ge)
                nc.vector.tensor_tensor(out=B[:, :ns * P].rearrange("p (c e) -> p c e", e=P),
                                        in0=iota_ce,
                                        in1=meta_en[:, s0:s0 + ns].broadcast_to([P, ns, P]),
                                        op=Alu.is_ge)
                te_b = wpool.tile([P, G * P], f16, tag="te_s", name="te_b", bufs=3)
                nc.vector.tensor_tensor(out=te_b[:],
                                        in0=A[:, :ns * P], in1=B[:, :ns * P],
                                        op=Alu.subtract)
                te_writes[t] = nc.sync.dma_start(out=te_dram[t], in_=te_b[:])
                # scatter one-hot -> DRAM
                et_b = wpool.tile([P, G * P], f16, tag="et", name="et_b", bufs=3)
                nc.vector.tensor_tensor(
                    out=et_b[:].rearrange("p (c e) -> p c e", e=P),
                    in0=meta_tr[:, t * G:(t + 1) * G].broadcast_to([P, G, P]),
                    in1=_mid_bcast(iota_bf[:], 1, G),
                    op=Alu.is_equal)
                et_writes[t] = nc.sync.dma_start(out=et_dram[t], in_=et_b[:])

            # ---- iterations
            agg_by_g = {}
            for it in range(niter):
                for t in range(NT):
                    zbuf = wpool.tile([P, G * d], f16, tag="zbuf", bufs=3)
                    ri = nc.sync.dma_start(out=zbuf[:], in_=z[t])
                    tile.add_dep_helper(ri.ins, z_writes[t].ins, reason="z ready")
                    et_t = wpool.tile([P, G * P], f16, tag="et", bufs=3)
                    er = nc.sync.dma_start(out=et_t[:], in_=et_dram[t])
                    tile.add_dep_helper(er.ins, et_writes[t].ins, reason="et ready")
                    te_t = wpool.tile([P, G * P], f16, tag="te_s", bufs=3)
                    tr_ = nc.sync.dma_start(out=te_t[:], in_=te_dram[t])
                    tile.add_dep_helper(tr_.ins, te_writes[t].ins, reason="te ready")

                    utrg = wpool.tile([P, G * d], f16, tag="utrg", bufs=3)
                    for h in range(G // 8):
                        ups = ppool.tile([P, 8 * d], f32, space="PSUM", tag="ups",
                                         name="ups", bufs=2)
                        for q in range(8):
                            qq = h * 8 + q
                            c = t * G + qq
                            s = int(ch_slot[c])
                            nc.tensor.matmul(
                                out=ups[:, q * d:(q + 1) * d],
                                lhsT=te_t[:, qq * P:(qq + 1) * P],
                                rhs=u_g[s // 4][:, (s % 4) * d:(s % 4 + 1) * d],
                                start=True, stop=True)
                        nc.scalar.activation(utrg[:, h * 8 * d:(h + 1) * 8 * d],
                                             ups[:], Act.Copy)

                    w = wpool.tile([P, G * d], f16, tag="w", bufs=3)
                    nc.vector.tensor_tensor(out=w[:], in0=zbuf[:], in1=utrg[:], op=Alu.mult)
                    lg = wpool.tile([P, G * K], f32, tag="lg", bufs=3)
                    nc.vector.tensor_reduce(
                        out=lg[:],
                        in_=_swap_last2(w[:].rearrange("p (c s k) -> p c s k", s=dd, k=K)),
                        axis=X, op=Alu.add)
                    ex = wpool.tile([P, G * K], f16, tag="ex", bufs=3)
                    nc.scalar.activation(ex[:], lg[:], Act.Exp)
                    sm = wpool.tile([P, G], f32, tag="sm", bufs=3)
                    nc.vector.tensor_reduce(out=sm[:],
                                            in_=ex[:].rearrange("p (c k) -> p c k", k=K),
                                            axis=X, op=Alu.add)
                    rq = wpool.tile([P, G], f16, tag="rq", bufs=3)
                    with nc.allow_low_precision(reason="softmax denom fits fp16"):
                        nc.vector.reciprocal(out=rq[:], in_=sm[:])
                    en = wpool.tile([P, G * K], f16, tag="en", bufs=3)
                    nc.vector.tensor_tensor(
                        out=en[:].rearrange("p (c k) -> p c k", k=K),
                        in0=ex[:].rearrange("p (c k) -> p c k", k=K),
                        in1=rq[:].rearrange("p c -> p c").broadcast_to([P, G, K]),
                        op=Alu.mult)
                    msg = wpool.tile([P, G * d], f16, tag="msg", bufs=3)
                    nc.vector.tensor_tensor(
                        out=msg[:].rearrange("p (c s k) -> p c s k", s=dd, k=K),
                        in0=zbuf[:].rearrange("p (c s k) -> p c s k", s=dd, k=K),
                        in1=_mid_bcast(en[:].rearrange("p (c k) -> p c k", k=K), 2, dd),
                        op=Alu.mult)

                    for q in range(G):
                        c = t * G + q
                        s = int(ch_slot[c])
                        g4 = s // 4
                        if c in slot_first and s % 4 == 0:
                            agg_by_g[g4] = apool.tile([P, 4 * d], f32, space="PSUM",
                                                      tag="agg", name="agg")
                        agg = agg_by_g[g4]
                        nc.tensor.matmul(
                            out=agg[:, (s % 4) * d:(s % 4 + 1) * d],
                            lhsT=et_t[:, q * P:(q + 1) * P],
                            rhs=msg[:, q * d:(q + 1) * d],
                            start=(c in slot_first), stop=(c in slot_last))
                        if c in slot_last and (s % 4 == 3 or s == NB - 1):
                            _emit_norm(nc, tile, wpool, g4, agg, xkl, xkl_writes[g4],
                                       u_g, u_out, mybir, Alu, Act, X, d, dd, epst)

    _bass_rust.move_matmul_waits_to_ldweights(nc.m)
    _bass_rust.generate_event_semaphores(nc)
    return nc


def _emit_norm(nc, tile, wpool, g4, agg, xkl, xkl_w, u_g, u_out, mybir, Alu, Act, X,
               d, dd, epst):
    f32 = mybir.dt.float32
    xkb = wpool.tile([P, 4 * d], f32, tag="xkb")
    ri = nc.sync.dma_start(out=xkb[:], in_=xkl[g4])
    tile.add_dep_helper(ri.ins, xkl_w.ins, reason="xkl ready")
    t0 = wpool.tile([P, 4 * d], f32, tag="t0")
    nc.vector.tensor_tensor(out=t0[:], in0=agg[:], in1=xkb[:], op=Alu.add)
    sq = wpool.tile([P, 4 * d], f32, tag="nsq")
    nc.vector.tensor_tensor(out=sq[:], in0=t0[:], in1=t0[:], op=Alu.mult)
    ssn = wpool.tile([P, 4 * K], f32, tag="nss")
    nc.vector.tensor_reduce(
        out=ssn[:],
        in_=_swap_last2(sq[:].rearrange("p (r s k) -> p r s k", s=dd, k=K)),
        axis=X, op=Alu.add)
    nrm = wpool.tile([P, 4 * K], f32, tag="nnr")
    nc.scalar.activation(nrm[:], ssn[:], Act.Sqrt, bias=epst[:])
    rsn = wpool.tile([P, 4 * K], f32, tag="nrs")
    nc.vector.reciprocal(out=rsn[:], in_=nrm[:])
    nc.vector.tensor_tensor(
        out=u_g[g4][:].rearrange("p (r s k) -> p r s k", s=dd, k=K),
        in0=t0[:].rearrange("p (r s k) -> p r s k", s=dd, k=K),
        in1=_mid_bcast(rsn[:].rearrange("p (r k) -> p r k", k=K), 2, dd),
        op=Alu.mult)
    nc.gpsimd.dma_start(
        out=u_out[g4 * 4 * P:(g4 + 1) * 4 * P].rearrange("(b p) f -> p b f", p=P),
        in_=u_g[g4][:])


# ----------------------------------------------------------------------------
# kernel entry
# ----------------------------------------------------------------------------

_CACHE = {}


def kernel(x: np.ndarray, edge_index: np.ndarray) -> np.ndarray:
    import ml_dtypes
    from concourse.bass_utils import run_bass_kernel_spmd

    x = np.asarray(x, dtype=np.float32)
    edge_index = np.asarray(edge_index)
    n, d = x.shape
    n_cores = 8
    lay = build_layout(edge_index, n, n_cores)

    key = (n, d, edge_index.shape[1], lay.C_tot, tuple(lay.caps))
    if key not in _CACHE:
        _CACHE[key] = build_nc(lay, niter=NITER, d=d)
    nc = _CACHE[key]

    NBG = math.ceil(lay.NB / 4)
    dd = d // K
    j = np.arange(d)
    perm = (j % K) * dd + (j // K)          # device col j  <- canonical col perm[j]
    perm_inv = np.empty(d, dtype=np.int64)
    perm_inv[perm] = j                       # canonical col i  <- device col perm_inv... 
    xp_ = x[:, perm]
    xb = np.vstack([xp_, np.ones((1, d), dtype=np.float32)])
    in_maps = []
    for c in range(n_cores):
        cl = lay.cores[c]
        xs = xp_[cl.src_idx]                                # [P, C_tot, d]
        x_src = np.ascontiguousarray(
            xs.reshape(P, lay.NT, lay.G, d).transpose(1, 0, 2, 3)
            .reshape(lay.NT, P, lay.G * d)).astype(np.float16)
        sl = np.full((P, NBG * 4), n, dtype=np.int64)
        sl[:, :lay.NB] = cl.slot_idx
        x_local = np.ascontiguousarray(
            xb[sl].reshape(P, NBG, 4, d).transpose(1, 0, 2, 3).reshape(NBG, P, 4 * d))
        in_maps.append({
            "x_src": x_src,
            "x_local": x_local,
            "trgrel": cl.trgrel.astype(ml_dtypes.bfloat16),
            "starts": cl.starts.astype(ml_dtypes.bfloat16),
            "ends": cl.ends.astype(ml_dtypes.bfloat16),
        })

    res = run_bass_kernel_spmd(nc, in_maps, core_ids=list(range(n_cores)))

    out = np.zeros((n, d), dtype=np.float32)
    for c in range(n_cores):
        u = res.results[c]["u_out"][:, perm_inv]
        lo = c * lay.n_loc
        out[lo:lo + lay.n_loc] = u[lay.cores[c].out_rows]
    return out


if __name__ == "__main__":
    import os
    os.environ.setdefault("JAX_PLATFORMS", "cpu")
    import jax
    import jax.numpy as jnp

    def ref(x, edge_index, niter):
        n, d = x.shape
        k, ddv = K, d // K
        s, t = edge_index[0], edge_index[1]

        def fnorm(v):
            nr = jnp.linalg.norm(v, axis=-1, keepdims=True)
            return v / jnp.maximum(nr, 1e-12)

        xk = fnorm(x.reshape(n, k, ddv))
        zz = xk[s]
        u = xk
        for _ in range(niter):
            p = jax.nn.softmax(jnp.sum(zz * u[t], axis=2), axis=1)
            msg = (zz * p[:, :, None]).reshape(-1, d)
            agg = jax.ops.segment_sum(msg, t, num_segments=n)
            u = fnorm(agg.reshape(n, k, ddv) + xk)
        return u.reshape(n, d)

    rng = np.random.default_rng(0)
    n, m, d = 12800, 100000, 128
    x = rng.standard_normal((n, d), dtype=np.float32)
    ei = rng.integers(0, n, size=(2, m)).astype(np.int64)
    want = np.asarray(ref(jnp.asarray(x), jnp.asarray(ei), NITER))
    got = numpy_model(x, ei, n_cores=8, fp16=True)
    err = np.abs(got - want).max() / np.abs(want).max()
    print(f"numpy model vs ref: fp16-err={err:.2e}")



# revision 27
# speedup vs baseline: 1.5916x; 1.5916x over previous
"""DisenConv (disentangled GNN message passing) Trainium2 kernel, v2.

Strategy (8 NeuronCores, no collectives):
  - Nodes split into 8 contiguous ranges; core c owns range c. Edges are
    assigned to the core owning their TARGET node and sorted by target, so
    both the u[trg] gather and the segment-sum scatter are core-local.
  - Per core, targets are grouped into 128-node blocks (slots, sorted by
    edge count); each slot's edges are padded into chunks of 128 edges.
    Gather and scatter are TensorE matmuls against per-chunk one-hot
    matrices built on the HOST and shipped as fp8 (exact 0/1):
      * et (scatter one-hot, [edge, target]) lives RESIDENT in SBUF.
      * te (gather one-hot, [target, edge]) streams from DRAM per tile.
  - x_src (raw, unnormalized, f16) streams from DRAM every iteration; the
    per-edge factor norms (nrm) are computed once on-device in iteration 1
    and kept resident; normalization folds into the small [P,256] softmax
    ops (logits/nrm, denom*nrm) instead of materializing z.
  - Per-tile engine split: gather/scatter on PE, PSUM evict + Square/Sqrt/
    Exp on ACT, big elementwise (w, msg) on DVE at 2x, the grouped logit
    reduction as a masked segmented scan on GPSIMD (Pool), softmax smalls
    on DVE using divide (no reciprocal).

kernel(**inputs) takes the FULL inputs and returns the FULL output.
"""
import math
from dataclasses import dataclass, field

import numpy as np

P = 128          # partitions / block size / chunk size
K = 8            # latent factors
NITER = 6
EPS2 = 1e-24     # sqrt(ss + EPS2) ~= max(sqrt(ss), 1e-12)


# ----------------------------------------------------------------------------
# host-side layout
# ----------------------------------------------------------------------------

@dataclass
class CoreLayout:
    src_idx: np.ndarray    # [P, C_tot] int32  source node of edge (chunk, lane); pad -> 0
    trgrel: np.ndarray     # [P, C_tot] float32 target-in-block; pad -> -1
    slot_idx: np.ndarray   # [P, NB] int32  node id of (slot block, lane); pad -> n_nodes
    out_rows: np.ndarray   # [n_loc] int32  u row of local node i


@dataclass
class Layout:
    n_nodes: int
    n_cores: int
    n_loc: int
    NB: int                # block slots per core
    caps: list             # chunks per slot (uniform across cores)
    cum: np.ndarray
    C_tot: int
    G: int                 # chunks per tile
    NT: int                # tiles
    cores: list = field(default_factory=list)


def build_layout(edge_index: np.ndarray, n_nodes: int, n_cores: int, G: int = 32) -> Layout:
    src = edge_index[0].astype(np.int64)
    trg = edge_index[1].astype(np.int64)
    assert n_nodes % n_cores == 0
    n_loc = n_nodes // n_cores
    NB = math.ceil(n_loc / P)

    per_core = []
    counts_all = np.zeros((n_cores, NB), dtype=np.int64)
    for c in range(n_cores):
        lo, hi = c * n_loc, (c + 1) * n_loc
        m = (trg >= lo) & (trg < hi)
        es, et = src[m], trg[m] - lo
        order = np.argsort(et, kind="stable")
        es, et = es[order], et[order]
        counts_all[c] = np.bincount(et // P, minlength=NB)
        per_core.append((es, et))

    orders = [np.argsort(-counts_all[c], kind="stable") for c in range(n_cores)]
    sorted_counts = np.stack([counts_all[c][orders[c]] for c in range(n_cores)])
    caps = np.maximum(1, np.ceil(sorted_counts.max(axis=0) / P).astype(np.int64))
    C_tot = int(caps.sum())
    pad = (-C_tot) % G
    caps[-1] += pad
    C_tot += pad
    cum = np.concatenate([[0], np.cumsum(caps)])[:-1]

    lay = Layout(n_nodes=n_nodes, n_cores=n_cores, n_loc=n_loc, NB=NB,
                 caps=[int(x) for x in caps], cum=cum, C_tot=C_tot, G=G, NT=C_tot // G)

    for c in range(n_cores):
        es, et = per_core[c]
        order_c = orders[c]
        slot_of_block = np.empty(NB, dtype=np.int64)
        slot_of_block[order_c] = np.arange(NB)

        src_idx = np.zeros((P, C_tot), dtype=np.int32)
        trgrel = np.full((P, C_tot), -1.0, dtype=np.float32)

        bstart = np.concatenate([[0], np.cumsum(counts_all[c])])
        for s in range(NB):
            b = order_c[s]
            e0, e1 = bstart[b], bstart[b + 1]
            n_e = e1 - e0
            if n_e == 0:
                continue
            tr = (et[e0:e1] - b * P).astype(np.int64)
            c0 = lay.cum[s]
            j = np.arange(n_e)
            src_idx[j % P, c0 + j // P] = es[e0:e1]
            trgrel[j % P, c0 + j // P] = tr

        slot_idx = np.full((P, NB), n_nodes, dtype=np.int32)
        for s in range(NB):
            b = order_c[s]
            base = c * n_loc + b * P
            n_in = min(P, n_loc - b * P)
            slot_idx[:n_in, s] = base + np.arange(n_in)

        i = np.arange(n_loc)
        out_rows = (slot_of_block[i // P] * P + (i % P)).astype(np.int32)

        lay.cores.append(CoreLayout(src_idx=src_idx, trgrel=trgrel,
                                    slot_idx=slot_idx, out_rows=out_rows))
    return lay


def chunk_slots(lay: Layout) -> np.ndarray:
    ch_slot = np.zeros(lay.C_tot, dtype=np.int64)
    for s in range(lay.NB):
        ch_slot[lay.cum[s]:lay.cum[s] + lay.caps[s]] = s
    return ch_slot


# ----------------------------------------------------------------------------
# numpy model of exactly what the device computes (for validation)
# ----------------------------------------------------------------------------

def numpy_model(x: np.ndarray, edge_index: np.ndarray, n_cores: int,
                niter: int = NITER) -> np.ndarray:
    def f16(v):
        return v.astype(np.float16)

    n, d = x.shape
    dd = d // K
    lay = build_layout(edge_index, n, n_cores)
    C, NB = lay.C_tot, lay.NB
    NBG = math.ceil(NB / 4)
    ch_slot = chunk_slots(lay)

    out = np.zeros((n, d), dtype=np.float32)
    for c in range(n_cores):
        cl = lay.cores[c]
        xs = f16(x[cl.src_idx])                     # [P, C, d]
        xb = np.vstack([x, np.ones((1, d), dtype=np.float32)])
        sl = np.full((P, NBG * 4), n, dtype=np.int64)
        sl[:, :NB] = cl.slot_idx
        xloc = f16(xb[sl])                          # [P, NBG*4, d]
        trgrel = cl.trgrel
        valid = trgrel >= 0
        tr = np.where(valid, trgrel, 0).astype(np.int64)

        # phase 0: xkl + u init
        sq = f16(xloc * xloc)
        ssn = f16(sq.reshape(P, NBG * 4, K, dd).astype(np.float32).sum(axis=3))
        nrml = f16(np.sqrt(ssn.astype(np.float32) + EPS2))
        rnl = f16(1.0 / nrml.astype(np.float32))
        xkl = f16(xloc.reshape(P, NBG * 4, K, dd) * rnl[..., None])
        u = xkl.reshape(P, NBG * 4, d).copy()

        # iter-1 per-edge factor norms
        sqe = f16(xs * xs)
        sse = f16(sqe.reshape(P, C, K, dd).astype(np.float32).sum(axis=3))
        nrm = f16(np.sqrt(sse.astype(np.float32) + EPS2))
        inm = f16(1.0 / nrm.astype(np.float32))             # [P, C, K]

        uo = None
        for _ in range(niter):
            utrg = np.zeros((P, C, d), dtype=np.float32)
            for ci in range(C):
                utrg[:, ci, :] = u[:, ch_slot[ci], :][tr[:, ci]]
            utrg *= valid[:, :, None]
            utrg = f16(utrg)
            w = f16(xs * utrg)
            lgraw = f16(w.reshape(P, C, K, dd).astype(np.float32).sum(axis=3))
            lg = f16(lgraw * inm)
            ex = f16(np.exp(lg.astype(np.float32)))
            sm = f16(ex.astype(np.float32).sum(axis=2))
            rq = f16(1.0 / sm.astype(np.float32))
            en1 = f16(ex * inm)
            en = f16(en1 * rq[:, :, None])
            msg = f16(xs.reshape(P, C, K, dd) * en[..., None])
            agg = np.zeros((P, NBG * 4, d), dtype=np.float32)
            msgf = msg.reshape(P, C, d).astype(np.float32) * valid[:, :, None]
            for ci in range(C):
                np.add.at(agg[:, ch_slot[ci], :], tr[:, ci], msgf[:, ci, :])
            t0 = f16(agg + xkl.reshape(P, NBG * 4, d).astype(np.float32))
            sqn = f16(t0 * t0)
            ssn2 = f16(sqn.reshape(P, NBG * 4, K, dd).astype(np.float32).sum(axis=3))
            nrm2 = f16(np.sqrt(ssn2.astype(np.float32) + EPS2))
            rn2 = f16(1.0 / nrm2.astype(np.float32))
            u = f16(t0.reshape(P, NBG * 4, K, dd) * rn2[..., None]).reshape(P, NBG * 4, d)
            uo = u
        lo = c * lay.n_loc
        urows = uo.transpose(1, 0, 2).reshape(-1, d)
        out[lo:lo + lay.n_loc] = urows[cl.out_rows]
    return out


# ----------------------------------------------------------------------------
# AP helpers
# ----------------------------------------------------------------------------

def _swap_last2(ap):
    import concourse.bass as bass
    lst = [list(x) for x in ap.ap]
    lst[-1], lst[-2] = lst[-2], lst[-1]
    return bass.AP(ap.tensor, ap.offset, lst)


def _mid_bcast(ap, pos, count):
    import concourse.bass as bass
    lst = [list(x) for x in ap.ap]
    lst.insert(pos, [0, count])
    return bass.AP(ap.tensor, ap.offset, lst)


def _ends(ap, esize=None):
    """Drop the last dim of `ap`, offset to its final element.

    AP offsets are in ELEMENTS (esize arg kept for call-site compat)."""
    import concourse.bass as bass
    lst = [list(x) for x in ap.ap]
    step, count = lst[-1]
    return bass.AP(ap.tensor, ap.offset + step * (count - 1), lst[:-1])


def _dim_slice(ap, dim, lo, hi, esize=None):
    """Slice dim `dim` (negative ok) of `ap` to [lo, hi). Element offsets."""
    import concourse.bass as bass
    lst = [list(x) for x in ap.ap]
    step, _ = lst[dim]
    lst[dim] = [step, hi - lo]
    return bass.AP(ap.tensor, ap.offset + step * lo, lst)


# ----------------------------------------------------------------------------
# bass kernel emitter
# ----------------------------------------------------------------------------

def build_nc(lay: Layout, niter: int = NITER, d: int = 128,
             TREE_DVE_CHUNKS: int = 32,
             MSG_POOL_QUARTERS: tuple = (3,),
             SQN_ON_DVE: bool = False,
             XS_BUFS: int = 5,
             UTRG_BUFS: int = 3,
             MSG_BUFS: int = 3,
             W_BUFS: int = 2,
             AGG_BUFS: int = 4,
             UPS_BUFS: int = 2,
             EB_BUFS: int = 4,
             SSN_POOL_TREE: bool = False,
             W_POOL_QUARTERS: tuple = (),
             T0_ON_POOL: bool = False,
             LOOKAHEAD: int = 2):
    import contextlib
    import concourse.bass as bass
    import concourse.mybir as mybir
    import concourse.tile as tile
    import bass_rust as _bass_rust

    f32, f16 = mybir.dt.float32, mybir.dt.float16
    fp8 = mybir.dt.float8e4
    Alu = mybir.AluOpType
    Act = mybir.ActivationFunctionType
    X = mybir.AxisListType.X
    dd = d // K

    G, C_tot, NT, NB = lay.G, lay.C_tot, lay.NT, lay.NB
    NBG = math.ceil(NB / 4)
    ch_slot = chunk_slots(lay)
    slot_first = {int(lay.cum[s]): s for s in range(NB)}
    slot_last = {int(lay.cum[s] + lay.caps[s] - 1): s for s in range(NB)}

    nc = bass.Bass()
    x_src = nc.dram_tensor("x_src", [NT, P, G * d], f16, kind="ExternalInput")
    x_local = nc.dram_tensor("x_local", [NBG, P, 4 * d], f16, kind="ExternalInput")
    et_in = nc.dram_tensor("et_in", [NT, P, G * P], fp8, kind="ExternalInput")
    te_in = nc.dram_tensor("te_in", [NT, P, G * P], fp8, kind="ExternalInput")
    u_out = nc.dram_tensor("u_out", [NBG, P, 4 * d], f16, kind="ExternalOutput")

    with tile.TileContext(nc) as tc:
        with contextlib.ExitStack() as ctx:
            cpool = ctx.enter_context(tc.tile_pool(name="const", bufs=1))
            wpool = ctx.enter_context(tc.tile_pool(name="work", bufs=2))
            ppool = ctx.enter_context(tc.tile_pool(name="psum", bufs=2, space="PSUM"))
            apool = ctx.enter_context(tc.tile_pool(name="aggp", bufs=AGG_BUFS, space="PSUM"))

            inm = cpool.tile([P, C_tot * K], f16, tag="inm")
            epst = cpool.tile([P, 1], f32, tag="epst")
            u_g = [cpool.tile([P, 4 * d], f16, tag=f"u{g}", name=f"u{g}")
                   for g in range(NBG)]
            xkl_g = [cpool.tile([P, 4 * d], f16, tag=f"xk{g}", name=f"xk{g}")
                     for g in range(NBG)]

            nc.vector.memset(epst[:], EPS2)

            def emit_tree(src_tile, nch, out_tile, dve_ch, tags):
                """Pairwise-add tree: [P, nch*dd*K] (k-inner) -> [P, nch*K]
                factor sums. Chunks [0, dve_ch) on DVE, rest on Pool."""
                scratch = [wpool.tile([P, nch * swid * K], f16,
                                      tag=f"{tags}{i}", name=f"{tags}{i}",
                                      bufs=3)
                           for i, swid in enumerate((8, 4, 2))]
                parts = []
                if dve_ch > 0:
                    parts.append((nc.vector, 0, dve_ch))
                if dve_ch < nch:
                    parts.append((nc.gpsimd, dve_ch, nch))
                lvl_in = src_tile[:].rearrange("p (c s k) -> p c s k",
                                               s=dd, k=K)
                for i, swid in enumerate((8, 4, 2, 1)):
                    ov_full = (out_tile if swid == 1 else scratch[i])[:] \
                        .rearrange("p (c s k) -> p c s k", s=swid, k=K)
                    for eng, c0, c1 in parts:
                        eng.tensor_tensor(
                            out=_dim_slice(ov_full, 1, c0, c1, 2),
                            in0=_dim_slice(_dim_slice(lvl_in, -2, 0, swid, 2),
                                           1, c0, c1, 2),
                            in1=_dim_slice(_dim_slice(lvl_in, -2, swid,
                                                      2 * swid, 2),
                                           1, c0, c1, 2),
                            op=Alu.add)
                    lvl_in = ov_full

            # ---- phase 0: xkl = rownorm(x_local); u init
            for g in range(NBG):
                xlt = wpool.tile([P, 4 * d], f16, tag="xlt", name="xlt")
                nc.sync.dma_start(out=xlt[:], in_=x_local[g])
                sql = wpool.tile([P, 4 * d], f16, tag="sql", name="sql")
                nc.scalar.activation(sql[:], xlt[:], Act.Square)
                ssl = wpool.tile([P, 4 * K], f16, tag="ssl", name="ssl")
                with nc.allow_low_precision(reason="factor sq-sums fit fp16"):
                    nc.vector.tensor_reduce(
                        out=ssl[:],
                        in_=_swap_last2(sql[:].rearrange(
                            "p (r s k) -> p r s k", s=dd, k=K)),
                        axis=X, op=Alu.add)
                nrml = wpool.tile([P, 4 * K], f16, tag="nrml", name="nrml")
                nc.scalar.activation(nrml[:], ssl[:], Act.Sqrt, bias=epst[:])
                rnl = wpool.tile([P, 4 * K], f16, tag="rnl", name="rnl")
                with nc.allow_low_precision(reason="unit-scale reciprocal"):
                    nc.vector.reciprocal(out=rnl[:], in_=nrml[:])
                nc.vector.tensor_tensor(
                    out=xkl_g[g][:].rearrange("p (r s k) -> p r s k", s=dd, k=K),
                    in0=xlt[:].rearrange("p (r s k) -> p r s k", s=dd, k=K),
                    in1=_mid_bcast(rnl[:].rearrange("p (r k) -> p r k", k=K), 2, dd),
                    op=Alu.mult)
                nc.vector.tensor_copy(out=u_g[g][:], in_=xkl_g[g][:])

            # ---- iterations
            agg_by_g = {}

            def emit_norm(g4, agg, it):
                t0 = wpool.tile([P, 4 * d], f16, tag="t0", name="t0", bufs=2)
                t0eng = nc.gpsimd if T0_ON_POOL else nc.vector
                t0eng.tensor_tensor(out=t0[:], in0=agg[:], in1=xkl_g[g4][:],
                                    op=Alu.add)
                sqn = wpool.tile([P, 4 * d], f16, tag="sqn", name="sqn", bufs=2)
                if SQN_ON_DVE:
                    nc.vector.tensor_tensor(out=sqn[:], in0=t0[:], in1=t0[:],
                                            op=Alu.mult)
                else:
                    nc.scalar.activation(sqn[:], t0[:], Act.Square)
                ssn = wpool.tile([P, 4 * K], f16, tag="ssn", name="ssn", bufs=3)
                if SSN_POOL_TREE:
                    emit_tree(sqn, 4, ssn, 0, "trn")
                else:
                    with nc.allow_low_precision(reason="factor sq-sums fit fp16"):
                        nc.vector.tensor_reduce(
                            out=ssn[:],
                            in_=_swap_last2(sqn[:].rearrange(
                                "p (r s k) -> p r s k", s=dd, k=K)),
                            axis=X, op=Alu.add)
                nrm2 = wpool.tile([P, 4 * K], f16, tag="nrm2", name="nrm2",
                                  bufs=3)
                nc.scalar.activation(nrm2[:], ssn[:], Act.Sqrt, bias=epst[:])
                rn2 = wpool.tile([P, 4 * K], f16, tag="rn2", name="rn2", bufs=3)
                with nc.allow_low_precision(reason="unit-scale reciprocal"):
                    nc.vector.reciprocal(out=rn2[:], in_=nrm2[:])
                nc.vector.tensor_tensor(
                    out=u_g[g4][:].rearrange("p (r s k) -> p r s k", s=dd, k=K),
                    in0=t0[:].rearrange("p (r s k) -> p r s k", s=dd, k=K),
                    in1=_mid_bcast(rn2[:].rearrange("p (r k) -> p r k", k=K), 2, dd),
                    op=Alu.mult)
                if it == niter - 1:
                    nc.sync.dma_start(out=u_out[g4], in_=u_g[g4][:])

            def emit_load_gather(it, t):
                xs = wpool.tile([P, G * d], f16, tag="xs", name="xs", bufs=XS_BUFS)
                nc.sync.dma_start(out=xs[:], in_=x_src[t])
                tb = wpool.tile([P, G * P], fp8, tag="tb", name="tb", bufs=3)
                nc.scalar.dma_start(out=tb[:], in_=te_in[t])
                eb = wpool.tile([P, G * P], fp8, tag="eb", name="eb", bufs=EB_BUFS)
                nc.scalar.dma_start(out=eb[:], in_=et_in[t])

                if it == 0:
                    # per-edge factor norms via pairwise tree
                    sqe = wpool.tile([P, G * d], f16, tag="utrg", name="sqe",
                                     bufs=UTRG_BUFS)
                    nc.scalar.activation(sqe[:], xs[:], Act.Square)
                    ssE = wpool.tile([P, G * K], f16, tag="ssE", name="ssE",
                                     bufs=3)
                    emit_tree(sqe, G, ssE, G, "trl")
                    nrmE = wpool.tile([P, G * K], f16, tag="nrmE", name="nrmE",
                                      bufs=3)
                    nc.scalar.activation(nrmE[:], ssE[:], Act.Sqrt, bias=epst[:])
                    with nc.allow_low_precision(reason="unit-scale reciprocal"):
                        nc.vector.reciprocal(
                            out=inm[:, t * G * K:(t + 1) * G * K], in_=nrmE[:])

                utrg = wpool.tile([P, G * d], f16, tag="utrg", name="utrg",
                                  bufs=UTRG_BUFS)
                for h in range(G // 8):
                    ups = ppool.tile([P, 8 * d], f32, space="PSUM", tag="ups",
                                     name="ups", bufs=UPS_BUFS)
                    for q in range(8):
                        qq = h * 8 + q
                        c = t * G + qq
                        s = int(ch_slot[c])
                        nc.tensor.matmul(
                            out=ups[:, q * d:(q + 1) * d],
                            lhsT=tb[:, qq * P:(qq + 1) * P],
                            rhs=u_g[s // 4][:, (s % 4) * d:(s % 4 + 1) * d],
                            start=True, stop=True)
                    nc.scalar.activation(utrg[:, h * 8 * d:(h + 1) * 8 * d],
                                         ups[:], Act.Copy)
                return xs, eb, utrg

            def emit_mid(it, t, xs, utrg):
                w = wpool.tile([P, G * d], f16, tag="w", name="w", bufs=W_BUFS)
                WQ = G // 4
                for wq in range(4):
                    sl = slice(wq * WQ * d, (wq + 1) * WQ * d)
                    eng = nc.gpsimd if wq in W_POOL_QUARTERS else nc.vector
                    eng.tensor_tensor(out=w[:, sl], in0=xs[:, sl],
                                      in1=utrg[:, sl], op=Alu.mult)
                # grouped factor sums via pairwise tree (DVE/Pool split)
                lgr = wpool.tile([P, G * K], f16, tag="lgr", name="lgr", bufs=3)
                emit_tree(w, G, lgr, TREE_DVE_CHUNKS, "trl")
                return lgr

            def emit_tail(it, t, xs, eb, lgr):
                inm_t = inm[:, t * G * K:(t + 1) * G * K]
                lg = wpool.tile([P, G * K], f16, tag="lg", name="lg", bufs=3)
                nc.vector.tensor_tensor(out=lg[:], in0=lgr[:], in1=inm_t,
                                        op=Alu.mult)
                ex = wpool.tile([P, G * K], f16, tag="ex", name="ex", bufs=3)
                nc.scalar.activation(ex[:], lg[:], Act.Exp)
                sm = wpool.tile([P, G], f16, tag="sm", name="sm", bufs=3)
                with nc.allow_low_precision(reason="softmax denom in fp16"):
                    nc.vector.tensor_reduce(
                        out=sm[:],
                        in_=ex[:].rearrange("p (c k) -> p c k", k=K),
                        axis=X, op=Alu.add)
                rq = wpool.tile([P, G], f16, tag="rq", name="rq", bufs=3)
                with nc.allow_low_precision(reason="softmax denom fits fp16"):
                    nc.vector.reciprocal(out=rq[:], in_=sm[:])
                en1 = wpool.tile([P, G * K], f16, tag="en1", name="en1", bufs=2)
                nc.vector.tensor_tensor(out=en1[:], in0=ex[:], in1=inm_t,
                                        op=Alu.mult)
                en = wpool.tile([P, G * K], f16, tag="en", name="en", bufs=3)
                nc.vector.tensor_tensor(
                    out=en[:].rearrange("p (c k) -> p c k", k=K),
                    in0=en1[:].rearrange("p (c k) -> p c k", k=K),
                    in1=_mid_bcast(rq[:].rearrange("p c -> p c"), 2, K),
                    op=Alu.mult)
                msg = wpool.tile([P, G * d], f16, tag="msg", name="msg",
                                 bufs=MSG_BUFS)
                MQ = G // 4
                for mq in range(4):
                    eng = nc.gpsimd if mq in MSG_POOL_QUARTERS else nc.vector
                    eng.tensor_tensor(
                        out=msg[:, mq * MQ * d:(mq + 1) * MQ * d]
                        .rearrange("p (c s k) -> p c s k", s=dd, k=K),
                        in0=xs[:, mq * MQ * d:(mq + 1) * MQ * d]
                        .rearrange("p (c s k) -> p c s k", s=dd, k=K),
                        in1=_mid_bcast(
                            en[:, mq * MQ * K:(mq + 1) * MQ * K]
                            .rearrange("p (c k) -> p c k", k=K), 2, dd),
                        op=Alu.mult)

                for q in range(G):
                    c = t * G + q
                    s = int(ch_slot[c])
                    g4 = s // 4
                    if c in slot_first and s % 4 == 0:
                        agg_by_g[g4] = apool.tile([P, 4 * d], f32,
                                                  space="PSUM", tag="agg",
                                                  name="agg")
                    agg = agg_by_g[g4]
                    nc.tensor.matmul(
                        out=agg[:, (s % 4) * d:(s % 4 + 1) * d],
                        lhsT=eb[:, q * P:(q + 1) * P],
                        rhs=msg[:, q * d:(q + 1) * d],
                        start=(c in slot_first), stop=(c in slot_last))
                    if c in slot_last and (s % 4 == 3 or s == NB - 1):
                        emit_norm(g4, agg, it)

            flat = [(it, t) for it in range(niter) for t in range(NT)]
            N = len(flat)
            loaded = {}
            for i in range(N + LOOKAHEAD):
                if i < N:
                    loaded[i] = emit_load_gather(*flat[i])
                k = i - LOOKAHEAD
                if 0 <= k < N:
                    xs, eb, utrg = loaded.pop(k)
                    lgr = emit_mid(*flat[k], xs, utrg)
                    emit_tail(*flat[k], xs, eb, lgr)
    _bass_rust.move_matmul_waits_to_ldweights(nc.m)
    _bass_rust.generate_event_semaphores(nc)
    return nc


# ----------------------------------------------------------------------------
# kernel entry
# ----------------------------------------------------------------------------

_CACHE = {}


def kernel(x: np.ndarray, edge_index: np.ndarray) -> np.ndarray:
    import ml_dtypes
    from concourse.bass_utils import run_bass_kernel_spmd

    x = np.asarray(x, dtype=np.float32)
    edge_index = np.asarray(edge_index)
    n, d = x.shape
    dd = d // K
    n_cores = 8
    lay = build_layout(edge_index, n, n_cores)
    NBG = math.ceil(lay.NB / 4)
    C, G, NT = lay.C_tot, lay.G, lay.NT

    key = (n, d, edge_index.shape[1], lay.C_tot, tuple(lay.caps))
    if key not in _CACHE:
        _CACHE[key] = build_nc(lay, niter=NITER, d=d)
    nc = _CACHE[key]

    j = np.arange(d)
    perm = (j % K) * dd + (j // K)          # device col j <- canonical col perm[j]
    perm_inv = np.empty(d, dtype=np.int64)
    perm_inv[perm] = j
    xp = x[:, perm]
    xb = np.vstack([xp, np.ones((1, d), dtype=np.float32)])

    lanes = np.arange(P)[:, None]
    in_maps = []
    for c in range(n_cores):
        cl = lay.cores[c]
        xs = xp[cl.src_idx].astype(np.float16)              # [P, C, d]
        x_src = np.ascontiguousarray(
            xs.reshape(P, NT, G, d).transpose(1, 0, 2, 3).reshape(NT, P, G * d))
        sl = np.full((P, NBG * 4), n, dtype=np.int64)
        sl[:, :lay.NB] = cl.slot_idx
        x_local = np.ascontiguousarray(
            xb[sl].reshape(P, NBG, 4, d).transpose(1, 0, 2, 3)
            .reshape(NBG, P, 4 * d)).astype(np.float16)

        trg = cl.trgrel.astype(np.int64)                    # [P, C]; -1 pad
        valid = cl.trgrel >= 0
        chn = np.broadcast_to(np.arange(C)[None, :], (P, C))
        lane = np.broadcast_to(lanes, (P, C))
        et = np.zeros((P, C * P), dtype=ml_dtypes.float8_e4m3)
        et[lane[valid], chn[valid] * P + trg[valid]] = 1.0
        et_in = np.ascontiguousarray(
            et.reshape(P, NT, G * P).transpose(1, 0, 2))
        te = np.zeros((P, C * P), dtype=ml_dtypes.float8_e4m3)
        te[trg[valid], chn[valid] * P + lane[valid]] = 1.0
        te_in = np.ascontiguousarray(
            te.reshape(P, NT, G * P).transpose(1, 0, 2))

        in_maps.append({
            "x_src": x_src,
            "x_local": x_local,
            "et_in": et_in,
            "te_in": te_in,
        })

    res = run_bass_kernel_spmd(nc, in_maps, core_ids=list(range(n_cores)))

    out = np.zeros((n, d), dtype=np.float32)
    for c in range(n_cores):
        uo = res.results[c]["u_out"].astype(np.float32)     # [NBG, P, 4*d]
        urows = (uo.reshape(NBG, P, 4, d).transpose(0, 2, 1, 3)
                 .reshape(NBG * 4 * P, d))
        lo = c * lay.n_loc
        out[lo:lo + lay.n_loc] = urows[lay.cores[c].out_rows][:, perm_inv]
    return out
